# revision 1
# baseline (speedup 1.0000x reference)
"""Trainium2 Bass kernel for DC+CE+self loss.

Fixed problem shape: feature [2,32,64,128,128] f32, net_output [2,2,64,128,128] f32,
target [2,1,64,128,128] int32. Output: scalar f32 loss.

Sharding: data-parallel over D (8 slices per core x 8 cores), halos provided
host-side for the dilation stages. Three SPMD launches with tiny host glue:
  L1: masked feature sums S[32], cnt, CE/dice partial sums.
  L2: per-voxel cosine sim vs normalized positive-mean (PE matmuls over a
      channel-composite layout), s_neg array, dilate(pos) ring partials,
      per-core top-256 candidates (gpsimd topk).
  L3: threshold at global 250th value, dilate hard-negative seeds, partials.
"""

import numpy as np

import concourse.bass as bass
import concourse.tile as tile
from concourse import bacc, mybir
from concourse import bass_utils
from concourse import library_config

B, C, D, H, W = 2, 32, 64, 128, 128
NCORES = 8
DS = D // NCORES              # 8 d-slices per core
NBV = DS * H * W              # 131072 voxels per (core, b)
PF = NBV // 128               # 1024 free elems per partition (natural tiles)
NG = 4                        # composite channel-groups
GV = NBV // NG                # 32768 voxels per group
NQ = 32                       # q index: 1024-voxel rows per group
CHF = 2048                    # composite chunk free size
NCHUNK = (NG * GV // 4) // CHF * 4 // NG  # chunks per b over j axis
HALO_F, HALO_B = 11, 10
DHALO = DS + HALO_F + HALO_B  # 29
WP = W + HALO_F + HALO_B      # 149
F32 = mybir.dt.float32
I32 = mybir.dt.int32
U32 = mybir.dt.uint32
AF = mybir.ActivationFunctionType
AL = mybir.AluOpType
AX = mybir.AxisListType
TOP_N = 250
NEG_BIG = 1e30
SMOOTH = 1e-5

_CACHE = {}


# ---------------------------------------------------------------- L1 ------
def build_l1():
    nc = bacc.Bacc("TRN2", target_bir_lowering=False, debug=False,
                   num_devices=NCORES)
    feat = nc.dram_tensor("feat", [B, C, DS, H, W], F32, kind="ExternalInput")
    net = nc.dram_tensor("net", [B, 2, DS, H, W], F32, kind="ExternalInput")
    lab = nc.dram_tensor("lab", [B, DS, H, W], I32, kind="ExternalInput")
    parta = nc.dram_tensor("part1a", [4, 16], F32, kind="ExternalOutput")
    partb = nc.dram_tensor("part1b", [1, 16], F32, kind="ExternalOutput")

    with tile.TileContext(nc) as tc:
        with tc.tile_pool(name="io", bufs=2) as io, \
             tc.tile_pool(name="fio", bufs=3) as fio, \
             tc.tile_pool(name="small", bufs=1) as small, \
             tc.tile_pool(name="psum", bufs=1, space="PSUM") as psum:
            accf = small.tile([128, 16], F32)
            nc.vector.memset(accf[:], 0.0)
            accc = small.tile([128, 16], F32)
            nc.vector.memset(accc[:], 0.0)
            scratch = small.tile([128, 4096], F32, tag="scr")

            posf = []
            posf_rep = []
            for b in range(B):
                pt = small.tile([128, PF], F32, tag=f"posf{b}")
                nc.gpsimd.dma_start(
                    pt[:], lab.ap()[b].rearrange("d h w -> (d h w)")
                    .rearrange("(p n) -> p n", p=128))
                posf.append(pt)
                pr = small.tile([128, 4096], F32, tag=f"posfrep{b}")
                for cs in range(4):
                    nc.sync.dma_start(pr[32 * cs:32 * cs + 32, :], pt[:])
                posf_rep.append(pr)

            # masked per-channel sums: 4-plane tiles, rows (cs, q32)
            for b in range(B):
                for cpl in range(8):
                    ft = fio.tile([128, 4096], F32, tag="feat")
                    nc.sync.dma_start(
                        ft[:],
                        feat.ap()[b, 4 * cpl:4 * cpl + 4]
                        .rearrange("c d h w -> c (d h w)")
                        .rearrange("c (q j) -> (c q) j", q=32))
                    nc.vector.affine_mul_reduce(
                        scratch[:], accf[:, b * 8 + cpl:b * 8 + cpl + 1],
                        ft[:], posf_rep[b][:], 1.0, 0.0)

            # CE / dice partials on net_output (cols: 0-1 cnt, 2-3 ln1,
            # 4-5 p1, 6-7 p1t, 8-9 tdx, 10-11 rl)
            for b in range(B):
                x0 = io.tile([128, PF], F32, tag="x0")
                x1 = io.tile([128, PF], F32, tag="x1")
                nc.sync.dma_start(
                    x0[:], net.ap()[b, 0].rearrange("d h w -> (d h w)")
                    .rearrange("(p n) -> p n", p=128))
                nc.sync.dma_start(
                    x1[:], net.ap()[b, 1].rearrange("d h w -> (d h w)")
                    .rearrange("(p n) -> p n", p=128))
                dx = io.tile([128, PF], F32, tag="dx")
                nc.gpsimd.tensor_tensor(dx[:], x1[:], x0[:], AL.subtract)
                ax_ = io.tile([128, PF], F32, tag="ax")
                nc.scalar.activation(ax_[:], dx[:], AF.Abs)
                ex = io.tile([128, PF], F32, tag="ex")
                nc.scalar.activation(ex[:], ax_[:], AF.Exp, scale=-1.0)
                ln1 = io.tile([128, PF], F32, tag="ln1")
                nc.scalar.activation(ln1[:], ex[:], AF.Ln, bias=1.0)
                rl = io.tile([128, PF], F32, tag="rl")
                nc.scalar.activation(rl[:], dx[:], AF.Relu)
                p1 = io.tile([128, PF], F32, tag="p1")
                nc.scalar.activation(p1[:], dx[:], AF.Sigmoid)
                nc.vector.reduce_sum(accc[:, 2 + b:3 + b], ln1[:], AX.X)
                nc.vector.reduce_sum(accc[:, 10 + b:11 + b], rl[:], AX.X)
                nc.vector.affine_mul_reduce(
                    scratch[:, 0:PF], accc[:, 8 + b:9 + b], dx[:],
                    posf[b][:], 1.0, 0.0)
                nc.vector.reduce_sum(accc[:, 4 + b:5 + b], p1[:], AX.X)
                nc.vector.affine_mul_reduce(
                    scratch[:, 0:PF], accc[:, 6 + b:7 + b], p1[:],
                    posf[b][:], 1.0, 0.0)
                nc.vector.reduce_sum(accc[:, 0 + b:1 + b], posf[b][:], AX.X)

            g4 = small.tile([128, 4], F32)
            nc.vector.memset(g4[:], 0.0)
            for cs in range(4):
                nc.vector.memset(g4[32 * cs:32 * cs + 32, cs:cs + 1], 1.0)
            ones = small.tile([128, 1], F32)
            nc.vector.memset(ones[:], 1.0)
            reda = psum.tile([4, 16], F32, tag="reda")
            nc.tensor.matmul(reda[:], g4[:], accf[:], start=True, stop=True)
            redb = psum.tile([1, 16], F32, tag="redb")
            nc.tensor.matmul(redb[:], ones[:], accc[:], start=True, stop=True)
            outa = small.tile([4, 16], F32)
            nc.vector.tensor_copy(outa[:], reda[:])
            outb = small.tile([1, 16], F32)
            nc.vector.tensor_copy(outb[:], redb[:])
            nc.sync.dma_start(parta.ap(), outa[:])
            nc.sync.dma_start(partb.ap(), outb[:])

    nc.compile()
    return nc


# ------------------------------------------------------- dilation bits ----
def _dilate_box(nc, pool, src, name):
    """Separable (W,D) box-count on a [128=H, DHALO*WP] padded tile.

    src: 0/1 data (0 in pads). Data w at padded offset [HALO_F, HALO_F+W).
    Output [128, DS*WP] box counts; output (d,w) at w offset [0, W).
    """
    # W prefix along the contiguous (d, w) stream
    pw = pool.tile([128, DHALO * WP], F32, tag=f"{name}_scan", name=f"{name}_pw")
    nc.vector.tensor_tensor_scan(pw[:], src[:], src[:], 0.0, AL.add, AL.bypass)
    # W box, written into (w, d)-transposed layout for the D scan
    bwt = pool.tile([128, WP * DHALO], F32, tag=f"{name}_big", name=f"{name}_bwt")
    nc.vector.memset(bwt[:], 0.0)
    vw = pw[:].rearrange("p (d w) -> p d w", w=WP)
    vbT = bwt[:].rearrange("p (w d) -> p d w", w=WP)
    nc.vector.tensor_tensor(vbT[:, :, 0:W], vw[:, :, 21:21 + W],
                            vw[:, :, 0:W], AL.subtract)
    # D prefix along the contiguous (w, d) stream
    pd = pool.tile([128, WP * DHALO], F32, tag=f"{name}_scan", name=f"{name}_pd")
    nc.vector.tensor_tensor_scan(pd[:], bwt[:], bwt[:], 0.0, AL.add, AL.bypass)
    # D box back into (d, w) layout
    out = pool.tile([128, DS * WP], F32, tag=f"{name}_bd")
    nc.vector.memset(out[:], 0.0)
    vout = out[:].rearrange("p (d w) -> p d w", w=WP)
    vpd = pd[:].rearrange("p (w d) -> p d w", w=WP)
    nc.vector.tensor_tensor(vout[:, 0:DS, 0:W], vpd[:, 21:21 + DS, 0:W],
                            vpd[:, 0:DS, 0:W], AL.subtract)
    return out


def _dilate_h(nc, pool, psum_pool, band, boxwd, name):
    """H band matmul + threshold: [128, DS*WP] 0/1 dilated mask."""
    free = DS * WP
    ps = psum_pool.tile([128, free], F32, tag=f"{name}_ps")
    off = 0
    while off < free:
        nn_ = min(512, free - off)
        nc.tensor.matmul(ps[:, off:off + nn_], band[:],
                         boxwd[:, off:off + nn_], start=True, stop=True)
        off += nn_
    dil = pool.tile([128, DS * W], F32, tag=f"{name}_dil")
    nc.vector.tensor_scalar(
        dil[:].rearrange("p (d w) -> p d w", w=W),
        ps[:].rearrange("p (d w) -> p d w", w=WP)[:, :, 0:W],
        0.5, None, AL.is_ge)
    return dil


# ---------------------------------------------------------------- L2 ------
def build_l2():
    nc = bacc.Bacc("TRN2", target_bir_lowering=False, debug=False,
                   num_devices=NCORES)
    feat = nc.dram_tensor("feat", [B, C, DS, H, W], F32, kind="ExternalInput")
    labh = nc.dram_tensor("labh", [B, DHALO, H, W], I32, kind="ExternalInput")
    ug = nc.dram_tensor("ug", [128, 288], F32, kind="ExternalInput")
    band = nc.dram_tensor("band", [128, 128], F32, kind="ExternalInput")
    sneg = nc.dram_tensor("sneg", [B, DS, H, W], F32, kind="ExternalOutput")
    cand = nc.dram_tensor("cand", [128, 32], U32, kind="ExternalOutput")
    part = nc.dram_tensor("part2", [1, 8], F32, kind="ExternalOutput")

    with tile.TileContext(nc) as tc:
        with tc.tile_pool(name="chunks", bufs=2) as chunks, \
             tc.tile_pool(name="sq", bufs=2) as sqp, \
             tc.tile_pool(name="post", bufs=1) as post, \
             tc.tile_pool(name="dil", bufs=1) as dilp, \
             tc.tile_pool(name="small", bufs=1) as small, \
             tc.tile_pool(name="mm", bufs=1, space="PSUM") as mmp, \
             tc.tile_pool(name="hps", bufs=1, space="PSUM") as hps:
            ugt = small.tile([128, 288], F32)
            nc.sync.dma_start(ugt[:], ug.ap())
            bandt = small.tile([128, 128], F32)
            nc.sync.dma_start(bandt[:], band.ap())
            accs = small.tile([128, 8], F32)
            nc.vector.memset(accs[:], 0.0)
            topk_in = nc.alloc_sbuf_tensor("topk_in", [128, 3136], F32).ap()
            nc.vector.memset(topk_in[:, 2048:3136], -NEG_BIG)

            nc.gpsimd.load_library(library_config.topk)

            for b in range(B):
                fb = feat.ap()[b].rearrange("c d h w -> c (d h w)")
                dot_ps = []
                nsq_ps = []
                for i in range(2):
                    dp = mmp.tile([128, 512], F32, tag=f"dot{i}",
                                  name=f"dot{i}_{b}")
                    dot_ps.append(dp)
                    nq = mmp.tile([128, 512], F32, tag=f"nsq{i}",
                                  name=f"nsq{i}_{b}")
                    nsq_ps.append(nq)
                for t in range(8):
                    ch = chunks.tile([128, 4096], F32, tag="ch")
                    nc.sync.dma_start(
                        ch[:],
                        fb[4 * t:4 * t + 4]
                        .rearrange("cs (g j) -> (cs g) j", g=32))
                    for half in range(2):
                        sq = sqp.tile([128, 2048], F32, tag="sq")
                        nc.scalar.activation(
                            sq[:], ch[:, half * 2048:(half + 1) * 2048],
                            AF.Square)
                        for shalf in range(2):
                            s = 2 * half + shalf
                            rows = slice(32 * s, 32 * s + 32)
                            for bk in range(2):
                                nc.tensor.matmul(
                                    nsq_ps[bk][rows, :], ugt[:, 256:288],
                                    sq[:, shalf * 1024 + bk * 512:
                                       shalf * 1024 + bk * 512 + 512],
                                    start=(t == 0), stop=(t == 7),
                                    tile_position=(0, 32 * s),
                                    skip_group_check=True)
                                nc.tensor.matmul(
                                    dot_ps[bk][rows, :],
                                    ugt[:, 32 * t:32 * t + 32],
                                    ch[:, s * 1024 + bk * 512:
                                       s * 1024 + bk * 512 + 512],
                                    start=(t == 0), stop=(t == 7),
                                    tile_position=(0, 32 * s),
                                    skip_group_check=True)
                # assemble [128, 1024]: row p=32s+g, free=bk*512+n
                dot_sb = post.tile([128, PF], F32, tag="dot_sb")
                nsq_sb = post.tile([128, PF], F32, tag="nsq_sb")
                for bk in range(2):
                    nc.vector.tensor_copy(dot_sb[:, bk * 512:(bk + 1) * 512],
                                          dot_ps[bk][:])
                    nc.vector.tensor_copy(nsq_sb[:, bk * 512:(bk + 1) * 512],
                                          nsq_ps[bk][:])
                # sim = dot / sqrt(nsq)
                rt = post.tile([128, PF], F32, tag="rt")
                nc.scalar.activation(rt[:], nsq_sb[:], AF.Sqrt)
                rcp = post.tile([128, PF], F32, tag="rcp")
                nc.vector.reciprocal(rcp[:], rt[:])
                sim = post.tile([128, PF], F32, tag="sim")
                nc.vector.tensor_tensor(sim[:], dot_sb[:], rcp[:], AL.mult)

                # posf in perm layout (row 4q+g <-> voxel g*GV + q*1024 + n)
                lperm = labh.ap()[b, HALO_F:HALO_F + DS] \
                    .rearrange("d h w -> (d h w)") \
                    .rearrange("(g s k n) -> s g k n",
                               g=32, s=4, k=2, n=512)
                pstage = post.tile([128, PF], I32, tag="pstage")
                nc.sync.dma_start(pstage[:], lperm)
                posf = post.tile([128, PF], F32, tag="posf")
                nc.vector.tensor_copy(posf[:], pstage[:])

                # pos partial: sum sim*posf
                scr = post.tile([128, PF], F32, tag="scr")
                nc.vector.affine_mul_reduce(
                    scr[:], accs[:, b:b + 1], sim[:], posf[:], 1.0, 0.0)

                # s_neg = sim - sim*posf - BIG*posf
                sn = post.tile([128, PF], F32, tag="sn")
                nc.vector.tensor_tensor(sn[:], sim[:], posf[:], AL.mult)
                nc.vector.tensor_tensor(sn[:], sim[:], sn[:], AL.subtract)
                nc.vector.scalar_tensor_tensor(sn[:], posf[:], -NEG_BIG,
                                               sn[:], AL.mult, AL.add)
                snd = sneg.ap()[b].rearrange("d h w -> (d h w)")
                nc.sync.dma_start(
                    snd.rearrange("(g s k n) -> s g k n",
                                  g=32, s=4, k=2, n=512), sn[:])
                # feed topk input (flat order, any permutation is fine)
                nc.sync.dma_start(topk_in[:, b * 1024:(b + 1) * 1024], sn[:])

                # ---- mis ring: dilate(posf) in [H,(d,w)] layout ----
                lstage = dilp.tile([128, DHALO * W], I32, tag="mis_scan", name="lstage")
                nc.sync.dma_start(
                    lstage[:], labh.ap()[b].rearrange("d h w -> h d w"))
                ph = dilp.tile([128, DHALO * WP], F32, tag="mis_big", name="ph")
                nc.vector.memset(ph[:], 0.0)
                vph = ph[:].rearrange("p (d w) -> p d w", w=WP)
                nc.vector.tensor_copy(
                    vph[:, :, HALO_F:HALO_F + W],
                    lstage[:].rearrange("p (d w) -> p d w", w=W))
                boxwd = _dilate_box(nc, dilp, ph, "mis")
                dil = _dilate_h(nc, dilp, hps, bandt, boxwd, "mis")
                # sneg in [H,(d,w)] natural layout (no pads needed here)
                sdil = dilp.tile([128, DS * W], F32, tag="sdil")
                nc.sync.dma_start(sdil[:],
                                  sneg.ap()[b].rearrange("d h w -> h d w"))
                vsd = sdil[:].rearrange("p (d w) -> p d w", w=W)
                relu_s = dilp.tile([128, DS * W], F32, tag="relu_s")
                nc.scalar.activation(relu_s[:], sdil[:], AF.Relu)
                negm = dilp.tile([128, DS * W], F32, tag="negm")
                nc.vector.tensor_scalar(negm[:], sdil[:], -1e29, None,
                                        AL.is_gt)
                dscr = dilp.tile([128, DS * W], F32, tag="sdil", name="dscr")
                nc.vector.affine_mul_reduce(
                    dscr[:], accs[:, 2 + b:3 + b], relu_s[:], dil[:],
                    1.0, 0.0)
                nc.vector.affine_mul_reduce(
                    dscr[:], accs[:, 4 + b:5 + b], negm[:], dil[:],
                    1.0, 0.0)

            # topk: one call, 8 tokens of vocab 50176 (real data padded
            # with -BIG filler to 3136 cols/partition)
            ctile = nc.alloc_sbuf_tensor("ctile", [128, 32], U32).ap()
            nc.gpsimd.topk(ctile, topk_in, tokens=8, vocab_size=50176, k=256)
            nc.sync.dma_start(cand.ap(), ctile)

            ones = small.tile([128, 1], F32)
            nc.vector.memset(ones[:], 1.0)
            red = hps.tile([1, 8], F32, tag="red")
            nc.tensor.matmul(red[:], ones[:], accs[:], start=True, stop=True)
            out_sb = small.tile([1, 8], F32)
            nc.vector.tensor_copy(out_sb[:], red[:])
            nc.sync.dma_start(part.ap(), out_sb[:])

    nc.compile()
    return nc


# ---------------------------------------------------------------- L3 ------
def build_l3():
    nc = bacc.Bacc("TRN2", target_bir_lowering=False, debug=False,
                   num_devices=NCORES)
    snegh = nc.dram_tensor("snegh", [B, DHALO, H, W], F32,
                           kind="ExternalInput")
    thr = nc.dram_tensor("thr", [128, 1], F32, kind="ExternalInput")
    band = nc.dram_tensor("band", [128, 128], F32, kind="ExternalInput")
    part = nc.dram_tensor("part3", [1, 4], F32, kind="ExternalOutput")

    with tile.TileContext(nc) as tc:
        with tc.tile_pool(name="dil", bufs=1) as dilp, \
             tc.tile_pool(name="small", bufs=1) as small, \
             tc.tile_pool(name="hps", bufs=1, space="PSUM") as hps:
            bandt = small.tile([128, 128], F32)
            nc.sync.dma_start(bandt[:], band.ap())
            thr_col = small.tile([128, 1], F32)
            nc.sync.dma_start(thr_col[:], thr.ap())
            accs = small.tile([128, 4], F32)
            nc.vector.memset(accs[:], 0.0)

            for b in range(B):
                sh = dilp.tile([128, DHALO * WP], F32, tag="sh")
                nc.vector.memset(sh[:], -NEG_BIG)
                vsh = sh[:].rearrange("p (d w) -> p d w", w=WP)
                nc.sync.dma_start(vsh[:, :, HALO_F:HALO_F + W],
                                  snegh.ap()[b].rearrange("d h w -> h d w"))
                hsm = dilp.tile([128, DHALO * WP], F32, tag="fin_big", name="hsm")
                nc.vector.tensor_scalar(hsm[:], sh[:], thr_col[:, 0:1], None,
                                        AL.is_ge)
                boxwd = _dilate_box(nc, dilp, hsm, "fin")
                dil = _dilate_h(nc, dilp, hps, bandt, boxwd, "fin")
                vint = vsh[:, HALO_F:HALO_F + DS, HALO_F:HALO_F + W]
                relu_s = dilp.tile([128, DS * W], F32, tag="relu_s")
                vrelu = relu_s[:].rearrange("p (d w) -> p d w", w=W)
                nc.scalar.activation(vrelu, vint, AF.Relu)
                negm = dilp.tile([128, DS * W], F32, tag="negm")
                vneg = negm[:].rearrange("p (d w) -> p d w", w=W)
                nc.vector.tensor_scalar(vneg, vint, -1e29, None, AL.is_gt)
                dscr = dilp.tile([128, DS * W], F32, tag="sdil", name="dscr")
                nc.vector.affine_mul_reduce(
                    dscr[:], accs[:, b:b + 1], relu_s[:], dil[:], 1.0, 0.0)
                nc.vector.affine_mul_reduce(
                    dscr[:], accs[:, 2 + b:3 + b], negm[:], dil[:], 1.0, 0.0)

            ones = small.tile([128, 1], F32)
            nc.vector.memset(ones[:], 1.0)
            red = hps.tile([1, 4], F32, tag="red")
            nc.tensor.matmul(red[:], ones[:], accs[:], start=True, stop=True)
            out_sb = small.tile([1, 4], F32)
            nc.vector.tensor_copy(out_sb[:], red[:])
            nc.sync.dma_start(part.ap(), out_sb[:])

    nc.compile()
    return nc


# ------------------------------------------------------------- driver ------
def _get(name, builder):
    if name not in _CACHE:
        _CACHE[name] = builder()
    return _CACHE[name]


def make_ug(std_n):
    # lhsT blocks: partition p = c_sub*32 + g.  Block t (cols t*32..t*32+32):
    # U_t[p, m] = std_n[4*t + c_sub] * (g == m).  Block 8 (cols 256..288): ones.
    ug = np.zeros((128, 288), np.float32)
    for cs in range(4):
        for g in range(32):
            p = cs * 32 + g
            for t in range(8):
                ug[p, t * 32 + g] = std_n[4 * t + cs]
            ug[p, 256 + g] = 1.0
    return ug


def make_band():
    band = np.zeros((128, 128), np.float32)
    for i in range(128):
        band[i, max(0, i - 10):i + 11] = 1.0
    return band


def kernel(feature, net_output, target):
    feature = np.ascontiguousarray(feature, dtype=np.float32)
    net_output = np.ascontiguousarray(net_output, dtype=np.float32)
    target = np.ascontiguousarray(target, dtype=np.int32)
    lab = target[:, 0]                                   # [B, D, H, W]

    cores = list(range(NCORES))

    # ---------------- L1 ----------------
    nc1 = _get("l1", build_l1)
    in1 = []
    for k in cores:
        d0 = k * DS
        in1.append({
            "feat": feature[:, :, d0:d0 + DS],
            "net": net_output[:, :, d0:d0 + DS],
            "lab": lab[:, d0:d0 + DS],
        })
    r1 = bass_utils.run_bass_kernel_spmd(nc1, in1, core_ids=cores)
    pa = np.stack([r1.results[k]["part1a"] for k in cores]).astype(np.float64)
    pb = np.stack([r1.results[k]["part1b"][0] for k in cores]).astype(np.float64)
    ta = pa.sum(axis=0)          # [4 cs, 16 (b*8+cpl)]
    tb = pb.sum(axis=0)          # [16]
    S = np.empty(C, np.float64)
    for cpl in range(8):
        for cs in range(4):
            S[4 * cpl + cs] = ta[cs, cpl] + ta[cs, 8 + cpl]
    cnt = tb[0] + tb[1]
    ce_sum = tb[2] + tb[3] + tb[10] + tb[11] - tb[8] - tb[9]
    sp1 = tb[4] + tb[5]
    sp1t = tb[6] + tb[7]

    NVOX = B * D * H * W
    ce = ce_sum / NVOX
    tp1 = sp1t
    fp1 = sp1 - sp1t
    fn1 = cnt - sp1t
    dc1 = (2.0 * tp1 + SMOOTH) / (2.0 * tp1 + fp1 + fn1 + SMOOTH + 1e-8)
    dice = -dc1

    std = S / max(cnt, 1.0)
    if cnt <= 0:
        std = np.zeros_like(std)
    nrm = float(np.sqrt((std * std).sum()))
    std_n = (std / max(nrm, 1e-12)).astype(np.float32)

    # ---------------- L2 ----------------
    ug = make_ug(std_n)
    band = make_band()
    labh_full = np.pad(lab, ((0, 0), (HALO_F, HALO_B), (0, 0), (0, 0)))
    nc2 = _get("l2", build_l2)
    in2 = []
    for k in cores:
        d0 = k * DS
        in2.append({
            "feat": feature[:, :, d0:d0 + DS],
            "labh": labh_full[:, d0:d0 + DS + HALO_F + HALO_B],
            "ug": ug,
            "band": band,
        })
    r2 = bass_utils.run_bass_kernel_spmd(nc2, in2, core_ids=cores)
    p2 = np.stack([r2.results[k]["part2"][0] for k in cores]).astype(np.float64)
    t2 = p2.sum(axis=0)
    sum_sim_pos = t2[0] + t2[1]
    mis_num = t2[2] + t2[3]
    mis_cnt = t2[4] + t2[5]
    pos_loss = (cnt - sum_sim_pos) / max(cnt, 1.0) if cnt > 0 else 0.0
    mis_loss = mis_num / max(mis_cnt, 1.0) if mis_cnt > 0 else 0.0

    cands = np.concatenate([
        r2.results[k]["cand"][:, :16].reshape(-1).view(np.float32)
        for k in cores])
    cands = np.sort(cands)[::-1]
    tstar = np.float32(cands[TOP_N - 1])

    sneg_full = np.concatenate([r2.results[k]["sneg"] for k in cores], axis=1)

    # ---------------- L3 ----------------
    snegh_full = np.pad(sneg_full, ((0, 0), (HALO_F, HALO_B), (0, 0), (0, 0)),
                        constant_values=-NEG_BIG)
    nc3 = _get("l3", build_l3)
    in3 = []
    thr_in = np.full((128, 1), tstar, np.float32)
    for k in cores:
        d0 = k * DS
        in3.append({
            "snegh": snegh_full[:, d0:d0 + DS + HALO_F + HALO_B],
            "thr": thr_in,
            "band": band,
        })
    r3 = bass_utils.run_bass_kernel_spmd(nc3, in3, core_ids=cores)
    p3 = np.stack([r3.results[k]["part3"][0] for k in cores]).astype(np.float64)
    t3 = p3.sum(axis=0)
    neg_num = t3[0] + t3[1]
    neg_cnt = t3[2] + t3[3]
    neg_loss = neg_num / max(neg_cnt, 1.0) if neg_cnt > 0 else 0.0

    total = ce + dice + 5.0 * (pos_loss + mis_loss + neg_loss)
    return np.float32(total)



# revision 24
# speedup vs baseline: 1.7915x; 1.7915x over previous
"""Trainium2 Bass kernel for DC+CE+self loss.

Fixed problem shape: feature [2,32,64,128,128] f32, net_output [2,2,64,128,128] f32,
target [2,1,64,128,128] int32. Output: scalar f32 loss.

Sharding: data-parallel over D (8 d-planes per core x 8 cores), halos provided
host-side for the dilation stages. Three SPMD launches with tiny host glue:
  L1: one streaming pass over feature: masked channel sums S[32], per-voxel
      inverse norms (squares -> PE identity-matmul -> rsqrt), fp8 copy of the
      feature written via casting SWDGE DMA, CE/dice partials, and the
      dilate(pos) mis-ring mask (prefix-sum box counts + fp16 H-band matmul).
  L2: per-voxel cosine sim = (sum_c std8[c]*feat8[c,v]) * rsq[v] via fp8
      diag-weight matmuls, s_neg assembly, masked reductions, per-core
      top-256 candidates (gpsimd topk).
  L3: threshold at global 250th value, dilate hard-negative seeds, partials.
"""

import numpy as np
import ml_dtypes

import concourse.bass as bass
import concourse.tile as tile
from concourse import bacc, mybir
from concourse import bass_utils
from concourse import library_config

B, C, D, H, W = 2, 32, 64, 128, 128
NCORES = 8
DS = D // NCORES              # 8 d-slices per core
NBV = DS * H * W              # 131072 voxels per (core, b)
PF = NBV // 128               # 1024 free elems per partition (natural tiles)
HALO_F, HALO_B = 11, 10
DHALO = DS + HALO_F + HALO_B  # 29
WP = W + HALO_F + HALO_B      # 149
F32 = mybir.dt.float32
F16 = mybir.dt.float16
BF16 = mybir.dt.bfloat16
F8 = mybir.dt.float8e4
I8 = mybir.dt.int8
I32 = mybir.dt.int32
U32 = mybir.dt.uint32
AF = mybir.ActivationFunctionType
AL = mybir.AluOpType
AX = mybir.AxisListType
TOP_N = 250
NEG_BIG = 1e30
SMOOTH = 1e-5

_CACHE = {}


# ------------------------------------------------------- dilation bits ----
def _dilate_box(nc, pool, src, name):
    """Separable (W,D) box-count on a [128=H, DHALO*WP] padded f16 tile.

    src: 0/1 f16 data (0 in pads). Data w at padded offset [HALO_F, HALO_F+W).
    Output [128, DS*WP] f16 box counts (<= 609, exact in f16);
    output (d,w) at w offset [0, W).
    """
    # W prefix along the contiguous (d, w) stream
    pw = pool.tile([128, DHALO * WP], F16, tag=f"{name}_scan", name=f"{name}_pw")
    nc.vector.tensor_tensor_scan(pw[:], src[:], src[:], 0.0, AL.add, AL.bypass)
    # W box, written into (w, d)-transposed layout for the D scan
    bwt = pool.tile([128, WP * DHALO], F16, tag=f"{name}_big", name=f"{name}_bwt")
    # only the flat tail w >= W is never written below
    nc.vector.memset(bwt[:, W * DHALO:], 0.0)
    vw = pw[:].rearrange("p (d w) -> p d w", w=WP)
    vbT = bwt[:].rearrange("p (w d) -> p d w", w=WP)
    nc.vector.tensor_tensor(vbT[:, :, 0:W], vw[:, :, 21:21 + W],
                            vw[:, :, 0:W], AL.subtract)
    # D prefix along the contiguous (w, d) stream
    pd = pool.tile([128, WP * DHALO], F16, tag=f"{name}_scan", name=f"{name}_pd")
    nc.vector.tensor_tensor_scan(pd[:], bwt[:], bwt[:], 0.0, AL.add, AL.bypass)
    # D box back into (d, w) layout
    out = pool.tile([128, DS * WP], F16, tag=f"{name}_bd")
    vout = out[:].rearrange("p (d w) -> p d w", w=WP)
    vpd = pd[:].rearrange("p (w d) -> p d w", w=WP)
    nc.vector.memset(vout[:, :, W:WP], 0.0)
    nc.vector.tensor_tensor(vout[:, 0:DS, 0:W], vpd[:, 21:21 + DS, 0:W],
                            vpd[:, 0:DS, 0:W], AL.subtract)
    return out


def _dilate_h(nc, pool, psum_pool, band, boxwd, name):
    """H band matmul (f16) + threshold: [128, DS*W] 0/1 f32 dilated mask."""
    free = DS * WP
    ps = psum_pool.tile([128, free], F32, tag=f"{name}_ps")
    off = 0
    while off < free:
        nn_ = min(512, free - off)
        nc.tensor.matmul(ps[:, off:off + nn_], band[:],
                         boxwd[:, off:off + nn_], start=True, stop=True)
        off += nn_
    dil = pool.tile([128, DS * W], F32, tag=f"{name}_dil")
    nc.vector.tensor_scalar(
        dil[:].rearrange("p (d w) -> p d w", w=W),
        ps[:].rearrange("p (d w) -> p d w", w=WP)[:, :, 0:W],
        0.5, None, AL.is_ge)
    return dil


# ---------------------------------------------------------------- L1 ------
def build_l1():
    nc = bacc.Bacc("TRN2", target_bir_lowering=False, debug=False,
                   num_devices=NCORES)
    feat = nc.dram_tensor("feat", [B, C, DS, H, W], F32, kind="ExternalInput")
    net = nc.dram_tensor("net", [B, 2, DS, H, W], F32, kind="ExternalInput")
    labh = nc.dram_tensor("labh", [B, DHALO, H, W], I8, kind="ExternalInput")
    band = nc.dram_tensor("band", [128, 128], F16, kind="ExternalInput")
    diag = nc.dram_tensor("diag", [128, 128], BF16, kind="ExternalInput")
    parta = nc.dram_tensor("part1a", [1, 64], F32, kind="ExternalOutput")
    partb = nc.dram_tensor("part1b", [1, 16], F32, kind="ExternalOutput")
    feat8 = nc.dram_tensor("feat8", [B, 8, 128, 4096], F8,
                           kind="ExternalOutput")
    rsq = nc.dram_tensor("rsq", [B, 128, PF], F16, kind="ExternalOutput")
    dil8 = nc.dram_tensor("dil8", [B, NBV], I8, kind="ExternalOutput")

    with tile.TileContext(nc) as tc:
        with tc.tile_pool(name="cht", bufs=3) as chp, \
             tc.tile_pool(name="sq", bufs=2) as sqp, \
             tc.tile_pool(name="io", bufs=1) as io, \
             tc.tile_pool(name="dil", bufs=1) as dilp, \
             tc.tile_pool(name="small", bufs=1) as small, \
             tc.tile_pool(name="nsqp", bufs=1, space="PSUM") as nsqp, \
             tc.tile_pool(name="hps", bufs=1, space="PSUM") as hps:
            bandt = small.tile([128, 128], F16)
            nc.sync.dma_start(bandt[:], band.ap())
            diagt = small.tile([128, 128], BF16)
            nc.sync.dma_start(diagt[:], diag.ap())
            accf = small.tile([128, 64], F32)
            nc.vector.memset(accf[:], 0.0)
            accc = small.tile([128, 16], F32)
            nc.vector.memset(accc[:], 0.0)
            scratch = small.tile([128, PF], F32, tag="scr")

            # ---- mis ring first (fills pipeline warmup): dilate(posf) ----
            for b in range(B):
                lstage = dilp.tile([128, DHALO * W], I8, tag="mis_ls",
                                   name=f"lstage{b}")
                nc.sync.dma_start(
                    lstage[:], labh.ap()[b].rearrange("d h w -> h d w"))
                ph = dilp.tile([128, DHALO * WP], F16, tag="mis_big",
                               name=f"ph{b}")
                vph = ph[:].rearrange("p (d w) -> p d w", w=WP)
                nc.vector.memset(vph[:, :, 0:HALO_F], 0.0)
                nc.vector.memset(vph[:, :, HALO_F + W:WP], 0.0)
                nc.vector.tensor_copy(
                    vph[:, :, HALO_F:HALO_F + W],
                    lstage[:].rearrange("p (d w) -> p d w", w=W))
                boxwd = _dilate_box(nc, dilp, ph, "mis")
                dil = _dilate_h(nc, dilp, hps, bandt, boxwd, "mis")
                # write mask as int8 in flat natural order (cast by dma)
                nc.gpsimd.dma_start(
                    dil8.ap()[b].rearrange(
                        "(d h4 h3 w) -> (h4 h3) d w",
                        d=DS, h4=16, h3=8, w=W),
                    dil[:].rearrange("p (d w) -> p d w", w=W))

            for b in range(B):
                # positives mask (flat natural layout), cast i8 -> f32 by DMA
                posf = small.tile([128, PF], F32, tag="posf", name=f"posf{b}")
                nc.gpsimd.dma_start(
                    posf[:], labh.ap()[b, HALO_F:HALO_F + DS]
                    .rearrange("d h w -> (d h w)")
                    .rearrange("(p n) -> p n", p=128))

                nsq_ps = []
                for bk in range(2):
                    nq = nsqp.tile([128, 512], F32, tag=f"nsq{bk}",
                                   name=f"nsq{bk}_{b}")
                    nsq_ps.append(nq)

                fb = feat.ap()[b].rearrange("c d h w -> c (d h w)")
                for cg in range(8):
                    cht = chp.tile([128, 4096], F32, tag="ch")
                    nc.sync.dma_start(
                        cht[:].rearrange("p (c j) -> p c j", c=4),
                        fb[4 * cg:4 * cg + 4]
                        .rearrange("c (q j) -> q c j", q=128))
                    sq = sqp.tile([128, 4096], BF16, tag="sq")
                    nc.scalar.activation(sq[:], cht[:], AF.Square)
                    for ci in range(4):
                        c = 4 * cg + ci
                        nc.vector.affine_mul_reduce(
                            scratch[:, 0:PF], accf[:, b * 32 + c:b * 32 + c + 1],
                            cht[:, ci * 1024:ci * 1024 + 1024], posf[:],
                            1.0, 0.0)
                        for bk in range(2):
                            nc.tensor.matmul(
                                nsq_ps[bk][:, :], diagt[:],
                                sq[:, ci * 1024 + bk * 512:
                                   ci * 1024 + bk * 512 + 512],
                                start=(c == 0), stop=(c == 31),
                                skip_group_check=True)
                    # fp8 copy of the feature (cast by SWDGE dma)
                    nc.gpsimd.dma_start(feat8.ap()[b, cg], cht[:])

                # per-voxel norm ||feat_v|| in f16 (host computes scale/sqt)
                rtile = small.tile([128, PF], F16, tag="rtile")
                for bk in range(2):
                    nc.scalar.activation(rtile[:, bk * 512:bk * 512 + 512],
                                         nsq_ps[bk][:], AF.Sqrt)
                nc.sync.dma_start(rsq.ap()[b], rtile[:])

                # CE / dice partials on net_output (cols: 0-1 cnt, 2-3 ln1,
                # 4-5 p1, 6-7 p1t, 8-9 tdx, 10-11 rl)
                x0 = io.tile([128, PF], F32, tag="x0")
                x1 = io.tile([128, PF], F32, tag="x1")
                nc.sync.dma_start(
                    x0[:], net.ap()[b, 0].rearrange("d h w -> (d h w)")
                    .rearrange("(p n) -> p n", p=128))
                nc.sync.dma_start(
                    x1[:], net.ap()[b, 1].rearrange("d h w -> (d h w)")
                    .rearrange("(p n) -> p n", p=128))
                dx = io.tile([128, PF], F32, tag="dx")
                nc.gpsimd.tensor_tensor(dx[:], x1[:], x0[:], AL.subtract)
                ax_ = io.tile([128, PF], F32, tag="ax")
                nc.scalar.activation(ax_[:], dx[:], AF.Abs)
                ex = io.tile([128, PF], F32, tag="ex")
                nc.scalar.activation(ex[:], ax_[:], AF.Exp, scale=-1.0)
                ln1 = io.tile([128, PF], F32, tag="ln1")
                nc.scalar.activation(ln1[:], ex[:], AF.Ln, bias=1.0)
                rl = io.tile([128, PF], F32, tag="rl")
                nc.scalar.activation(rl[:], dx[:], AF.Relu)
                p1 = io.tile([128, PF], F32, tag="p1")
                nc.scalar.activation(p1[:], dx[:], AF.Sigmoid)
                nc.vector.reduce_sum(accc[:, 2 + b:3 + b], ln1[:], AX.X)
                nc.vector.reduce_sum(accc[:, 10 + b:11 + b], rl[:], AX.X)
                nc.vector.affine_mul_reduce(
                    scratch[:, 0:PF], accc[:, 8 + b:9 + b], dx[:],
                    posf[:], 1.0, 0.0)
                nc.vector.reduce_sum(accc[:, 4 + b:5 + b], p1[:], AX.X)
                nc.vector.affine_mul_reduce(
                    scratch[:, 0:PF], accc[:, 6 + b:7 + b], p1[:],
                    posf[:], 1.0, 0.0)
                nc.vector.reduce_sum(accc[:, 0 + b:1 + b], posf[:], AX.X)

            ones = small.tile([128, 1], F32)
            nc.vector.memset(ones[:], 1.0)
            red = hps.tile([1, 80], F32, tag="red")
            nc.tensor.matmul(red[:, 0:64], ones[:], accf[:],
                             start=True, stop=True)
            nc.tensor.matmul(red[:, 64:80], ones[:], accc[:],
                             start=True, stop=True)
            outa = small.tile([1, 64], F32)
            nc.vector.tensor_copy(outa[:], red[:, 0:64])
            outb = small.tile([1, 16], F32)
            nc.vector.tensor_copy(outb[:], red[:, 64:80])
            nc.sync.dma_start(parta.ap(), outa[:])
            nc.sync.dma_start(partb.ap(), outb[:])

    nc.compile()
    return nc


# ---------------------------------------------------------------- L2 ------
def build_l2():
    nc = bacc.Bacc("TRN2", target_bir_lowering=False, debug=False,
                   num_devices=NCORES)
    feat8 = nc.dram_tensor("feat8", [B, 8, 128, 4096], F8,
                           kind="ExternalInput")
    rsq = nc.dram_tensor("rsq", [B, 128, PF], F16, kind="ExternalInput")
    dil8 = nc.dram_tensor("dil8", [B, NBV], I8, kind="ExternalInput")
    lab8 = nc.dram_tensor("lab8", [B, DS, H, W], I8, kind="ExternalInput")
    ug8d = nc.dram_tensor("ug8d", [128, 32 * 128], F8, kind="ExternalInput")
    sneg = nc.dram_tensor("sneg", [B, NBV], F16, kind="ExternalOutput")
    cand = nc.dram_tensor("cand", [128, 32], U32, kind="ExternalOutput")
    part = nc.dram_tensor("part2", [1, 8], F32, kind="ExternalOutput")

    with tile.TileContext(nc) as tc:
        with tc.tile_pool(name="chunks", bufs=3) as chunks, \
             tc.tile_pool(name="post", bufs=1) as post, \
             tc.tile_pool(name="small", bufs=1) as small, \
             tc.tile_pool(name="mm", bufs=2, space="PSUM") as mmp, \
             tc.tile_pool(name="hps", bufs=1, space="PSUM") as hps:
            ugt = small.tile([128, 32 * 128], F8)
            nc.sync.dma_start(ugt[:], ug8d.ap())
            accs = small.tile([128, 8], F32)
            nc.vector.memset(accs[:], 0.0)
            scratch = small.tile([128, PF], F32, tag="scr")
            topk_in = nc.alloc_sbuf_tensor("topk_in", [128, 3136], F32).ap()
            nc.vector.memset(topk_in[:, 2048:3136], -NEG_BIG)

            nc.gpsimd.load_library(library_config.topk)

            for b in range(B):
                dot_ps = []
                for bk in range(2):
                    dp = mmp.tile([128, 512], F32, tag=f"dot{bk}",
                                  name=f"dot{bk}_{b}")
                    dot_ps.append(dp)
                for cg in range(8):
                    ch8 = chunks.tile([128, 4096], F8, tag="ch")
                    nc.sync.dma_start(ch8[:], feat8.ap()[b, cg])
                    for ci in range(4):
                        c = 4 * cg + ci
                        for bk in range(2):
                            nc.tensor.matmul(
                                dot_ps[bk][:, :],
                                ugt[:, c * 128:c * 128 + 128],
                                ch8[:, ci * 1024 + bk * 512:
                                    ci * 1024 + bk * 512 + 512],
                                start=(c == 0), stop=(c == 31),
                                skip_group_check=True)

                rsqt = post.tile([128, PF], F16, tag="rsqt")
                nc.sync.dma_start(rsqt[:], rsq.ap()[b])
                posf = post.tile([128, PF], F32, tag="posf")
                nc.gpsimd.dma_start(
                    posf[:], lab8.ap()[b].rearrange("d h w -> (d h w)")
                    .rearrange("(p n) -> p n", p=128))
                dilt = post.tile([128, PF], F32, tag="dilt")
                nc.gpsimd.dma_start(
                    dilt[:], dil8.ap()[b].rearrange("(p n) -> p n", p=128))

                dot_sb = post.tile([128, PF], F32, tag="dot_sb")
                for bk in range(2):
                    nc.vector.tensor_copy(dot_sb[:, bk * 512:(bk + 1) * 512],
                                          dot_ps[bk][:])
                sim = post.tile([128, PF], F32, tag="sim")
                nc.vector.tensor_tensor(sim[:], dot_sb[:], rsqt[:], AL.mult)

                # pos partial: sum sim*posf
                nc.vector.affine_mul_reduce(
                    scratch[:], accs[:, b:b + 1], sim[:], posf[:], 1.0, 0.0)

                # s_neg = sim - sim*posf - BIG*posf  (written into topk_in)
                tmp = post.tile([128, PF], F32, tag="tmp")
                nc.vector.tensor_tensor(tmp[:], sim[:], posf[:], AL.mult)
                nc.vector.tensor_tensor(tmp[:], sim[:], tmp[:], AL.subtract)
                snv = topk_in[:, b * 1024:(b + 1) * 1024]
                nc.vector.scalar_tensor_tensor(snv, posf[:], -NEG_BIG,
                                               tmp[:], AL.mult, AL.add)
                # f16 copy for L3 (cast by SWDGE dma; -1e30 -> -inf, fine)
                nc.gpsimd.dma_start(
                    sneg.ap()[b].rearrange("(p n) -> p n", p=128), snv)

                # mis-ring reductions vs precomputed dilated mask
                relu_s = post.tile([128, PF], F32, tag="relu_s")
                nc.scalar.activation(relu_s[:], snv, AF.Relu)
                negm = post.tile([128, PF], F32, tag="negm")
                nc.vector.tensor_scalar(negm[:], snv, -1e29, None, AL.is_gt)
                nc.vector.affine_mul_reduce(
                    scratch[:], accs[:, 2 + b:3 + b], relu_s[:], dilt[:],
                    1.0, 0.0)
                nc.vector.affine_mul_reduce(
                    scratch[:], accs[:, 4 + b:5 + b], negm[:], dilt[:],
                    1.0, 0.0)

            # topk: one call, 8 tokens of vocab 50176 (real data padded
            # with -BIG filler to 3136 cols/partition)
            ctile = nc.alloc_sbuf_tensor("ctile", [128, 32], U32).ap()
            nc.gpsimd.topk(ctile, topk_in, tokens=8, vocab_size=50176, k=256)
            nc.sync.dma_start(cand.ap(), ctile)

            ones = small.tile([128, 1], F32)
            nc.vector.memset(ones[:], 1.0)
            red = hps.tile([1, 8], F32, tag="red")
            nc.tensor.matmul(red[:], ones[:], accs[:], start=True, stop=True)
            out_sb = small.tile([1, 8], F32)
            nc.vector.tensor_copy(out_sb[:], red[:])
            nc.sync.dma_start(part.ap(), out_sb[:])

    nc.compile()
    return nc


# ---------------------------------------------------------------- L3 ------
def build_l3():
    nc = bacc.Bacc("TRN2", target_bir_lowering=False, debug=False,
                   num_devices=NCORES)
    snegh = nc.dram_tensor("snegh", [B, DHALO, H, W], F16,
                           kind="ExternalInput")
    thr = nc.dram_tensor("thr", [128, 1], F32, kind="ExternalInput")
    band = nc.dram_tensor("band", [128, 128], F16, kind="ExternalInput")
    part = nc.dram_tensor("part3", [1, 4], F32, kind="ExternalOutput")

    with tile.TileContext(nc) as tc:
        with tc.tile_pool(name="dil", bufs=1) as dilp, \
             tc.tile_pool(name="small", bufs=1) as small, \
             tc.tile_pool(name="hps", bufs=1, space="PSUM") as hps:
            bandt = small.tile([128, 128], F16)
            nc.sync.dma_start(bandt[:], band.ap())
            thr_col = small.tile([128, 1], F32)
            nc.sync.dma_start(thr_col[:], thr.ap())
            accs = small.tile([128, 4], F32)
            nc.vector.memset(accs[:], 0.0)

            for b in range(B):
                sh = dilp.tile([128, DHALO * WP], F16, tag="sh")
                vsh = sh[:].rearrange("p (d w) -> p d w", w=WP)
                nc.vector.memset(vsh[:, :, 0:HALO_F], -60000.0)
                nc.vector.memset(vsh[:, :, HALO_F + W:WP], -60000.0)
                nc.sync.dma_start(vsh[:, :, HALO_F:HALO_F + W],
                                  snegh.ap()[b].rearrange("d h w -> h d w"))
                hsm = dilp.tile([128, DHALO * WP], F16, tag="fin_big", name="hsm")
                nc.vector.tensor_scalar(hsm[:], sh[:], thr_col[:, 0:1], None,
                                        AL.is_ge)
                boxwd = _dilate_box(nc, dilp, hsm, "fin")
                dil = _dilate_h(nc, dilp, hps, bandt, boxwd, "fin")
                vint = vsh[:, HALO_F:HALO_F + DS, HALO_F:HALO_F + W]
                relu_s = dilp.tile([128, DS * W], F32, tag="relu_s")
                vrelu = relu_s[:].rearrange("p (d w) -> p d w", w=W)
                nc.scalar.activation(vrelu, vint, AF.Relu)
                negm = dilp.tile([128, DS * W], F32, tag="negm")
                vneg = negm[:].rearrange("p (d w) -> p d w", w=W)
                nc.vector.tensor_scalar(vneg, vint, -30000.0, None, AL.is_gt)
                dscr = dilp.tile([128, DS * W], F32, tag="sdil", name="dscr")
                nc.vector.affine_mul_reduce(
                    dscr[:], accs[:, b:b + 1], relu_s[:], dil[:], 1.0, 0.0)
                nc.vector.affine_mul_reduce(
                    dscr[:], accs[:, 2 + b:3 + b], negm[:], dil[:], 1.0, 0.0)

            ones = small.tile([128, 1], F32)
            nc.vector.memset(ones[:], 1.0)
            red = hps.tile([1, 4], F32, tag="red")
            nc.tensor.matmul(red[:], ones[:], accs[:], start=True, stop=True)
            out_sb = small.tile([1, 4], F32)
            nc.vector.tensor_copy(out_sb[:], red[:])
            nc.sync.dma_start(part.ap(), out_sb[:])

    nc.compile()
    return nc


# ------------------------------------------------------------- driver ------
def _get(name, builder):
    if name not in _CACHE:
        _CACHE[name] = builder()
    return _CACHE[name]


def make_band16():
    band = np.zeros((128, 128), np.float16)
    for i in range(128):
        band[i, max(0, i - 10):i + 11] = 1.0
    return band


def make_diag_bf16():
    return np.eye(128, dtype=ml_dtypes.bfloat16)


def make_ug8d(u8):
    # lhsT block per channel c: diag(u8[c]) in fp8, laid [128, 32*128]
    ug = np.zeros((128, 32 * 128), ml_dtypes.float8_e4m3)
    for c in range(32):
        for p in range(128):
            ug[p, c * 128 + p] = u8[c]
    return ug


def kernel(feature, net_output, target):
    feature = np.ascontiguousarray(feature, dtype=np.float32)
    net_output = np.ascontiguousarray(net_output, dtype=np.float32)
    target = np.ascontiguousarray(target, dtype=np.int32)
    lab = target[:, 0]                                   # [B, D, H, W]
    lab8 = lab.astype(np.int8)
    labh8 = np.pad(lab8, ((0, 0), (HALO_F, HALO_B), (0, 0), (0, 0)))
    band16 = make_band16()
    diagbf = make_diag_bf16()

    cores = list(range(NCORES))

    # ---------------- L1 ----------------
    nc1 = _get("l1", build_l1)
    in1 = []
    for k in cores:
        d0 = k * DS
        in1.append({
            "feat": feature[:, :, d0:d0 + DS],
            "net": net_output[:, :, d0:d0 + DS],
            "labh": labh8[:, d0:d0 + DS + HALO_F + HALO_B],
            "band": band16,
            "diag": diagbf,
        })
    r1 = bass_utils.run_bass_kernel_spmd(nc1, in1, core_ids=cores)
    pa = np.stack([r1.results[k]["part1a"][0] for k in cores]).astype(np.float64)
    pb = np.stack([r1.results[k]["part1b"][0] for k in cores]).astype(np.float64)
    ta = pa.sum(axis=0)          # [64] cols b*32 + c
    tb = pb.sum(axis=0)          # [16]
    S = ta[0:32] + ta[32:64]
    cnt = tb[0] + tb[1]
    ce_sum = tb[2] + tb[3] + tb[10] + tb[11] - tb[8] - tb[9]
    sp1 = tb[4] + tb[5]
    sp1t = tb[6] + tb[7]

    NVOX = B * D * H * W
    ce = ce_sum / NVOX
    tp1 = sp1t
    fp1 = sp1 - sp1t
    fn1 = cnt - sp1t
    dc1 = (2.0 * tp1 + SMOOTH) / (2.0 * tp1 + fp1 + fn1 + SMOOTH + 1e-8)
    dice = -dc1

    std = S / max(cnt, 1.0)
    if cnt <= 0:
        std = np.zeros_like(std)
    nrm = float(np.sqrt((std * std).sum()))
    std_n = (std / max(nrm, 1e-12)).astype(np.float32)
    # fp8 direction actually used on-device; fold its exact norm into rsq
    u8 = std_n.astype(ml_dtypes.float8_e4m3)
    un = u8.astype(np.float64)
    nrm8 = float(np.sqrt((un * un).sum()))
    scale = np.float32(1.0 / nrm8) if nrm8 > 0 else np.float32(0.0)
    ug8d = make_ug8d(u8)

    # ---------------- L2 ----------------
    nc2 = _get("l2", build_l2)
    in2 = []
    for k in cores:
        d0 = k * DS
        sqt = r1.results[k]["rsq"].astype(np.float32)   # ||feat_v|| in f16
        in2.append({
            "feat8": r1.results[k]["feat8"],
            "rsq": (scale / sqt).astype(np.float16),
            "dil8": r1.results[k]["dil8"],
            "lab8": lab8[:, d0:d0 + DS],
            "ug8d": ug8d,
        })
    r2 = bass_utils.run_bass_kernel_spmd(nc2, in2, core_ids=cores)
    p2 = np.stack([r2.results[k]["part2"][0] for k in cores]).astype(np.float64)
    t2 = p2.sum(axis=0)
    sum_sim_pos = t2[0] + t2[1]
    mis_num = t2[2] + t2[3]
    mis_cnt = t2[4] + t2[5]
    pos_loss = (cnt - sum_sim_pos) / max(cnt, 1.0) if cnt > 0 else 0.0
    mis_loss = mis_num / max(mis_cnt, 1.0) if mis_cnt > 0 else 0.0

    cands = np.concatenate([
        r2.results[k]["cand"][:, :16].reshape(-1).view(np.float32)
        for k in cores])
    cands = np.sort(cands)[::-1]
    tstar = np.float32(cands[TOP_N - 1])

    sneg_full = np.concatenate(
        [r2.results[k]["sneg"].reshape(B, DS, H, W) for k in cores], axis=1)

    # ---------------- L3 ----------------
    snegh_full = np.pad(sneg_full, ((0, 0), (HALO_F, HALO_B), (0, 0), (0, 0)),
                        constant_values=np.float16(-60000.0))
    nc3 = _get("l3", build_l3)
    in3 = []
    thr_in = np.full((128, 1), tstar, np.float32)
    for k in cores:
        d0 = k * DS
        in3.append({
            "snegh": snegh_full[:, d0:d0 + DS + HALO_F + HALO_B],
            "thr": thr_in,
            "band": band16,
        })
    r3 = bass_utils.run_bass_kernel_spmd(nc3, in3, core_ids=cores)
    p3 = np.stack([r3.results[k]["part3"][0] for k in cores]).astype(np.float64)
    t3 = p3.sum(axis=0)
    neg_num = t3[0] + t3[1]
    neg_cnt = t3[2] + t3[3]
    neg_loss = neg_num / max(neg_cnt, 1.0) if neg_cnt > 0 else 0.0

    total = ce + dice + 5.0 * (pos_loss + mis_loss + neg_loss)
    return np.float32(total)
